# revision 17
# baseline (speedup 1.0000x reference)
"""Trainium2 Bass kernel for nn_AutoregressiveDecoder.

Reference computation (B=2048, T=1024, D=32, S=2):
    s_{t+1} = s_t @ Ws.T + z_t @ Wz.T        (Ws = W[:, :2], Wz = W[:, 2:])
    out[:, t] = s_t,  s_0 = init_states      -> (B, T, S) fp32

Strategy: data-parallel over 8 cores (256 batch rows each). The sequential
scan is re-expressed as 16 chunks of C=64 steps; within a chunk all 64
outputs are produced by ONE PE accumulation group against a host-precomputed
block-Toeplitz matrix Q[(tl,d),(j,s)] = (Wz^T M^{j-1-tl})[d,s] (M = Ws^T),
plus a carry-in term via R[(s'),(j,s)] = M^j. The inter-chunk carry is a
16-step chain of tiny matmuls. Matmuls run in float32r (11-bit mantissa,
1 cycle/row at N=256); the carry chain uses hi/lo splitting so its error
stays at fp32 level. z is transposed on-chip with PE transposes (batch must
leave the partition dim for the contraction); output stays in (t, b) layout
and is transposed on the host.

Output rows use REVERSED j order (row = (C-1-j)*S + s) so that the last
state of a chunk lands on partitions 0:2 (partition bases must be 32-aligned
on this hardware).
"""

import numpy as np

B, T, D, S = 2048, 1024, 32, 2
C = 64                  # time steps per chunk
NCORES = 8
BL = B // NCORES        # 256 batch rows per core
KT = C * D // 128       # 16 K-tiles of 128 per chunk


# ---------------------------------------------------------------------------
# host-side helpers
# ---------------------------------------------------------------------------

def _rne11(v):
    """Round fp32 to 11 mantissa bits, round-to-nearest-even — bit-exact model
    of the TRN2 float32r rounding (verified on hardware)."""
    v = np.ascontiguousarray(v, np.float32)
    u = v.view(np.uint32)
    low = u & np.uint32(0xFFF)
    keep = u & np.uint32(0xFFFFF000)
    lsb = (u >> np.uint32(12)) & np.uint32(1)
    up = (low > 0x800) | ((low == 0x800) & (lsb == 1))
    r = keep + (up.astype(np.uint32) << np.uint32(12))
    return r.view(np.float32)


def _host_constants(W):
    """Build Q/R/M/u-last operand matrices from W (fp64 powers, fp32 out).

    Output column index m = (C-1-j)*S + s  (reversed j)."""
    W64 = W.astype(np.float64)
    M = W64[:, :S].T            # (2, 2)
    WzT = W64[:, S:].T          # (32, 2)

    Mpow = [np.eye(S)]
    for _ in range(C + 1):
        Mpow.append(Mpow[-1] @ M)

    def col(j):
        return (C - 1 - j) * S

    Q = np.zeros((C * D, C * S), np.float64)
    for j in range(1, C):
        for tl in range(j):
            Q[tl * D:(tl + 1) * D, col(j):col(j) + S] = WzT @ Mpow[j - 1 - tl]
    R = np.zeros((S, C * S), np.float64)
    for j in range(C):
        R[:, col(j):col(j) + S] = Mpow[j]

    Rf = R.astype(np.float32)
    R_hi = _rne11(Rf)
    R_lo = _rne11(Rf - R_hi)
    Mf = M.astype(np.float32)           # lhsT layout: lhsT[k=s', m=s] = M[s', s]
    M_hi = _rne11(Mf)
    M_lo = _rne11(Mf - M_hi)

    # M operands padded to M=128 output columns (only cols 0:2 meaningful)
    mhi2 = np.zeros((2, 128), np.float32)
    mhi2[:, 0:S] = M_hi
    mlo2 = np.zeros((2, 128), np.float32)
    mlo2[:, 0:S] = M_lo

    # u_last operand: full K=128 tile, only rows 96:128 (t-local 63) nonzero
    ulast = np.zeros((128, 128), np.float32)
    ulast[96:128, 0:S] = WzT.astype(np.float32)

    qmat = np.ascontiguousarray(
        Q.astype(np.float32).reshape(KT, 128, C * S))        # (16, 128, 128)

    return {
        "qmat": qmat,
        "rhi2": np.ascontiguousarray(R_hi),                   # (2, 128)
        "rlo2": np.ascontiguousarray(R_lo),                   # (2, 128)
        "mhi2": mhi2,
        "mlo2": mlo2,
        "ulast": ulast,
        "ident": np.eye(128, dtype=np.float32),
    }


# ---------------------------------------------------------------------------
# workarounds for this container's walrus (max 1 sem-wait per instruction)
# ---------------------------------------------------------------------------

def _install_patches():
    import concourse.tile as tile
    import concourse.mybir as mybir
    from bass_rust import ScopedClock

    if getattr(tile.TileContext, "_ard_patched", False):
        return

    def _drain_and_barrier(self, tick_clock, wait_clock):
        nc = self.nc
        probe = nc.sync.nop(nofuse=True, hint="tail_wait_spread")
        wait_clock.add_sem_waits(
            probe.ins, ScopedClock({None: tick_clock.global_clock})
        )
        si = probe.ins.sync_info
        waits = list(si.on_wait) if si is not None else []
        updates = list(si.on_update) if si is not None else []
        if len(waits) > 1:
            probe.ins.sync_info = mybir.SyncInfo(on_wait=waits[:1], on_update=updates)
            for w in waits[1:]:
                n2 = nc.sync.nop(nofuse=True, hint="tail_wait_spread")
                n2.ins.sync_info = mybir.SyncInfo(on_wait=[w], on_update=[])
        nc.sync.drain()
        nc.all_engine_barrier()
        assert self.sems is not None
        popped = nc._tile_sem_poison_stack.pop()
        assert popped is self._sem_poison
        nc.clear_and_free_semaphores(list(self.sems.allocated().values()))
        nc.all_engine_barrier()

    tile.TileContext._drain_and_barrier = _drain_and_barrier
    tile.TileContext._ard_patched = True


def _spread_waits(nc):
    """Move excess sem-waits (>1) onto same-engine NoOps inserted just before
    the owning instruction (engines are in-order, so semantics hold)."""
    import concourse.mybir as mybir

    ctr = 0
    for f in nc.m.functions:
        for b in f.blocks:
            out = []
            changed = False
            for inst in b.instructions:
                si = inst.sync_info
                waits = list(si.on_wait) if si is not None else []
                if len(waits) > 1 and inst.engine != mybir.EngineType.Unassigned:
                    changed = True
                    for w in waits[:-1]:
                        ctr += 1
                        out.append(
                            mybir.InstNoOp(
                                name=f"waitspread-{ctr}",
                                sync_info=mybir.SyncInfo(on_wait=[w], on_update=[]),
                                bass_nofuse=True,
                                engine=inst.engine,
                            )
                        )
                    inst.sync_info = mybir.SyncInfo(
                        on_wait=waits[-1:], on_update=list(si.on_update)
                    )
                out.append(inst)
            if changed:
                b.instructions = out
    return ctr


# ---------------------------------------------------------------------------
# device program
# ---------------------------------------------------------------------------

def _build_nc(nch):
    import concourse.bass as bass
    import concourse.tile as tile
    import concourse.mybir as mybir

    _install_patches()
    f32 = mybir.dt.float32
    f32r = mybir.dt.float32r
    PSUM = bass.MemorySpace.PSUM
    Tl = nch * C
    assert nch % 2 == 0
    NP = nch // 2                # chunk pairs; convs run at N=512 over a pair

    nc = bass.Bass(trn_type="TRN2", target_bir_lowering=False, debug=False)
    zin = nc.dram_tensor("zin", [BL, Tl, D], f32r, kind="ExternalInput")
    qmat = nc.dram_tensor("qmat", [KT, 128, C * S], f32r, kind="ExternalInput")
    rhi2 = nc.dram_tensor("rhi2", [2, C * S], f32r, kind="ExternalInput")
    rlo2 = nc.dram_tensor("rlo2", [2, C * S], f32r, kind="ExternalInput")
    mhi2 = nc.dram_tensor("mhi2", [2, 128], f32r, kind="ExternalInput")
    mlo2 = nc.dram_tensor("mlo2", [2, 128], f32r, kind="ExternalInput")
    ulast = nc.dram_tensor("ulast", [128, 128], f32r, kind="ExternalInput")
    ident = nc.dram_tensor("ident", [128, 128], f32r, kind="ExternalInput")
    c0hi = nc.dram_tensor("c0hi", [2, BL], f32r, kind="ExternalInput")
    c0lo = nc.dram_tensor("c0lo", [2, BL], f32r, kind="ExternalInput")
    out = nc.dram_tensor("out", [128, nch * BL], f32, kind="ExternalOutput")

    with tile.TileContext(nc) as tc:
        with (
            tc.tile_pool(name="const", bufs=1) as const,
            tc.tile_pool(name="zload", bufs=10) as zload,
            tc.tile_pool(name="ztsb", bufs=32) as ztsb,
            tc.tile_pool(name="obuf", bufs=2) as obuf,
            tc.tile_pool(name="chib", bufs=3) as chib,
            tc.tile_pool(name="clob", bufs=3) as clob,
            tc.tile_pool(name="ztps", bufs=4, space=PSUM) as ztps,
            tc.tile_pool(name="outps", bufs=3, space=PSUM) as outps,
            tc.tile_pool(name="cps", bufs=1, space=PSUM) as cpsp,
        ):
            qsb = const.tile([128, KT * C * S], f32r)
            nc.sync.dma_start(
                qsb[:].rearrange("p (k m) -> p k m", k=KT),
                qmat.ap().rearrange("k p m -> p k m"),
            )
            rhisb = const.tile([2, C * S], f32r)
            nc.sync.dma_start(rhisb[:], rhi2.ap())
            rlosb = const.tile([2, C * S], f32r)
            nc.sync.dma_start(rlosb[:], rlo2.ap())
            mhisb = const.tile([2, 128], f32r)
            nc.sync.dma_start(mhisb[:], mhi2.ap())
            mlosb = const.tile([2, 128], f32r)
            nc.sync.dma_start(mlosb[:], mlo2.ap())
            ulsb = const.tile([128, 128], f32r)
            nc.sync.dma_start(ulsb[:], ulast.ap())
            idsb = const.tile([128, 128], f32r)
            nc.sync.dma_start(idsb[:], ident.ap())

            chi = [None] * nch
            clo = [None] * nch
            chi[0] = chib.tile([2, BL], f32r, tag="chi", name="chi_t")
            nc.sync.dma_start(chi[0][:], c0hi.ap())
            clo[0] = clob.tile([2, BL], f32r, tag="clo", name="clo_t")
            nc.sync.dma_start(clo[0][:], c0lo.ap())

            # ------- helpers -------
            ncopy = [0]

            def dma_chunk(k):
                """Load z for one chunk: 2 x 1MB (one per batch group)."""
                zn = []
                for bg in range(2):
                    zt = zload.tile([128, C * D], f32r, tag="z", name="znat")
                    nc.sync.dma_start(
                        zt[:],
                        zin.ap()[bg * 128:(bg + 1) * 128,
                                 k * C:(k + 1) * C, :]
                        .rearrange("p t d -> p (t d)"),
                    )
                    zn.append(zt)
                return zn

            def tr_block(zn, ztiles, gs):
                """Transpose tile-groups `gs` (2 K-tiles each) of one chunk
                into per-chunk (td, b) sbuf tiles."""
                for g in gs:
                    ztp = ztps.tile([128, 512], f32r)
                    for h in range(2):
                        kt = 2 * g + h
                        for bg in range(2):
                            off = h * 256 + bg * 128
                            nc.tensor.transpose(
                                ztp[:, off:off + 128],
                                zn[bg][:, kt * 128:kt * 128 + 128],
                                idsb[:],
                            )
                    zsb = ztsb.tile([128, 512], f32r)
                    ncopy[0] += 1
                    if ncopy[0] % 8 < 3:
                        nc.vector.tensor_copy(zsb[:], ztp[:])
                    else:
                        nc.scalar.copy(zsb[:], ztp[:])
                    ztiles[g] = zsb

            def conv2(pout, zt, g):
                """The two conv matmuls of K-tile group g for one chunk. Each
                conv's f32r weight load (~290ns) hides behind the transpose
                matmuls interleaved just before it."""
                for h in range(2):
                    kt = 2 * g + h
                    nc.tensor.matmul(
                        pout[:], qsb[:, kt * 128:(kt + 1) * 128],
                        zt[g][:, h * 256:h * 256 + 256],
                        start=(kt == 0), stop=False,
                    )

            def carry_add(pout, k):
                nc.tensor.matmul(pout[:], rhisb[:], chi[k][:], start=False, stop=False)
                nc.tensor.matmul(pout[:], rhisb[:], clo[k][:], start=False, stop=False)
                nc.tensor.matmul(pout[:], rlosb[:], chi[k][:], start=False, stop=True)

            obs = {}

            def stag(pout, k):
                if k % 4 == 0:
                    obs[k // 4] = obuf.tile([128, 4 * BL], f32, tag="ob", name="ob_t")
                ob = obs[k // 4]
                col = (k % 4) * BL
                nc.scalar.copy(ob[:, col:col + BL], pout[:])
                return ob, col

            shilo = {}

            def build_shilo(k, ob, col):
                shi = chib.tile([2, BL], f32r, tag="chi", name="chi_t")
                nc.vector.tensor_copy(shi[:], ob[0:2, col:col + BL])
                slo = clob.tile([2, BL], f32r, tag="clo", name="clo_t")
                nc.vector.tensor_sub(slo[:], ob[0:2, col:col + BL], shi[:].bitcast(f32))
                shilo[k] = (shi, slo)

            def carry_upd_mms(k, zt15):
                """PE part of c_{k+1} = s_last(k) @ M + u_last(k)."""
                shi, slo = shilo[k]
                cp = cpsp.tile([128, BL], f32)
                nc.tensor.matmul(cp[:], mhisb[:], shi[:], start=True, stop=False)
                nc.tensor.matmul(cp[:], mhisb[:], slo[:], start=False, stop=False)
                nc.tensor.matmul(cp[:], mlosb[:], shi[:], start=False, stop=False)
                nc.tensor.matmul(cp[:], ulsb[:], zt15[:, 256:512],
                                 start=False, stop=True)
                return cp

            def build_chilo(k, cp):
                chi[k] = chib.tile([2, BL], f32r, tag="chi", name="chi_t")
                nc.vector.tensor_copy(chi[k][:], cp[0:2, :])
                clo[k] = clob.tile([2, BL], f32r, tag="clo", name="clo_t")
                nc.vector.tensor_sub(clo[k][:], cp[0:2, :], chi[k][:].bitcast(f32))

            # ------- prologue: pair 0 data -------
            ztA, ztB = {}, {}       # per-chunk zT tiles, current pair
            ztAn, ztBn = {}, {}     # next pair
            znA0 = dma_chunk(0)
            znB0 = dma_chunk(1)
            if NP > 1:
                znA1 = dma_chunk(2)
            tr_block(znA0, ztA, range(8))
            if NP > 1:
                znB1 = dma_chunk(3)
            tr_block(znB0, ztB, range(8))
            zn_n = (znA1, znB1) if NP > 1 else None

            # ------- steady-state pair loop -------
            # Per iteration: 16 interleaved units of [4 transposes (pair p+1)]
            # [2 convs (pair p)] so every conv weight-load hides behind the
            # preceding transpose matmuls; the cross-engine carry chain ops
            # are slotted at unit boundaries so PE never idles on them.
            zt_prev = None          # previous pair's chunk-B zT (for u_last)
            for p in range(NP):
                k0, k1 = 2 * p, 2 * p + 1
                last = p == NP - 1
                znA_f = znB_f = None
                if p + 2 <= NP - 1:
                    znA_f = dma_chunk(2 * p + 4)

                poutA = outps.tile([128, BL], f32, tag="pout", name="poutA")
                poutB = outps.tile([128, BL], f32, tag="pout", name="poutB")

                # --- units 0..7: transposes A(p+1) + convs A(p) ---
                for g in range(8):
                    if not last:
                        tr_block(zn_n[0], ztAn, [g])
                    conv2(poutA, ztA, g)
                    if g == 2 and p > 0:
                        # carry-update for chunk k0-1 (inputs from last iter)
                        cp = carry_upd_mms(k0 - 1, zt_prev[7])
                        build_chilo(k0, cp)

                carry_add(poutA, k0)
                ob, col = stag(poutA, k0)
                build_shilo(k0, ob, col)

                if p + 2 <= NP - 1:
                    znB_f = dma_chunk(2 * p + 5)

                # --- units 8..15: transposes B(p+1) + convs B(p) ---
                for g in range(8):
                    if not last:
                        tr_block(zn_n[1], ztBn, [g])
                    conv2(poutB, ztB, g)
                    if g == 3:
                        # carry-update chunk k0 -> c[k1]
                        cp = carry_upd_mms(k0, ztA[7])
                        build_chilo(k1, cp)

                carry_add(poutB, k1)
                ob, col = stag(poutB, k1)
                if not last:
                    build_shilo(k1, ob, col)
                    zt_prev = ztB

                if p % 2 == 1:
                    g = p // 2
                    nc.sync.dma_start(
                        out.ap()[:, g * 4 * BL:(g + 1) * 4 * BL], obs[g][:]
                    )

                ztA, ztB = ztAn, ztBn
                ztAn, ztBn = {}, {}
                zn_n = (znA_f, znB_f)

            if NP % 2 == 1:
                g = NP // 2
                nc.sync.dma_start(
                    out.ap()[:, g * 4 * BL:g * 4 * BL + 2 * BL],
                    obs[g][:, 0:2 * BL],
                )

    _spread_waits(nc)
    return nc


_CACHE = {}


def _get_nc(nch):
    if nch not in _CACHE:
        _CACHE[nch] = _build_nc(nch)
    return _CACHE[nch]


# ---------------------------------------------------------------------------
# entry point
# ---------------------------------------------------------------------------

def _run(init_states, z, W, nch, core_ids, trace=False):
    from concourse.bass_utils import run_bass_kernel_spmd

    consts = _host_constants(W)
    ncores = len(core_ids)
    in_maps = []
    for i in range(ncores):
        sl = slice(i * BL, (i + 1) * BL)
        init_T = np.ascontiguousarray(init_states[sl].T, np.float32)  # (2, BL)
        hi = _rne11(init_T)
        lo = _rne11(init_T - hi)
        in_maps.append({
            "zin": np.ascontiguousarray(z[sl, :nch * C, :], np.float32),
            "qmat": consts["qmat"],
            "rhi2": consts["rhi2"],
            "rlo2": consts["rlo2"],
            "mhi2": consts["mhi2"],
            "mlo2": consts["mlo2"],
            "ulast": consts["ulast"],
            "ident": consts["ident"],
            "c0hi": hi,
            "c0lo": lo,
        })

    nc = _get_nc(nch)
    kwargs = {}
    if trace:
        kwargs = dict(trace=True, trace_cores=list(core_ids))
    res = run_bass_kernel_spmd(nc, in_maps, core_ids=list(core_ids), **kwargs)

    outs = []
    for i in range(ncores):
        o = res.results[i]["out"]                       # (128, nch*BL)
        o = o.reshape(C, S, nch, BL)                    # (rev_j, s, k, b)
        o = o[::-1]                                     # undo reversed j
        o = np.transpose(o, (3, 2, 0, 1)).reshape(BL, nch * C, S)
        outs.append(o)
    full = np.concatenate(outs, axis=0).astype(np.float32)
    return full, res


def kernel(init_states, z, W):
    full, _ = _run(init_states, z, W, T // C, list(range(NCORES)))
    return full


# revision 18
# speedup vs baseline: 1.9827x; 1.9827x over previous
"""Trainium2 Bass kernel for nn_AutoregressiveDecoder.

Reference computation (B=2048, T=1024, D=32, S=2):
    s_{t+1} = s_t @ Ws.T + z_t @ Wz.T        (Ws = W[:, :2], Wz = W[:, 2:])
    out[:, t] = s_t,  s_0 = init_states      -> (B, T, S) fp32

Strategy: data-parallel over 8 cores (256 batch rows each). The sequential
scan is re-expressed as 16 chunks of C=64 steps; within a chunk all 64
outputs are produced by ONE PE accumulation group against a host-precomputed
block-Toeplitz matrix Q[(tl,d),(j,s)] = (Wz^T M^{j-1-tl})[d,s] (M = Ws^T),
plus a carry-in term. The inter-chunk carry is fused into the next chunk's
accumulation:  O_{k+1} += s_last(k)·(M R) + z_last(k)·(Wz^T R),  where
R[(s'),(j,s)] = M^j, so there is no separate carry matmul group at all.
The carry state is hi/lo-split in fp16 (scaled by 1/16 against overflow) so
the 16-step chain keeps fp32-level accuracy.

z is pre-transposed AND pre-converted to fp16 on the host (m10 ~ the PE's
native f32r m11 precision, half the HBM traffic), so the kernel needs no
on-chip transposes — all PE work is plain fp16 matmuls, which also keeps
the PE HAM clock-gate warm (transpose-mode ops do not count as PE activity
and previously kept the clock throttled at 1.2 GHz).

Output rows use REVERSED j order (row = (C-1-j)*S + s) so the last state of
a chunk lands on partitions 0:2 (partition bases must be 32-aligned).
"""

import numpy as np

B, T, D, S = 2048, 1024, 32, 2
C = 64                  # time steps per chunk
NCORES = 8
BL = B // NCORES        # 256 batch rows per core
KT = C * D // 128       # 16 K-tiles of 128 per chunk
CSC = 1.0 / 16.0        # carry scale (power of two; MR is stored x16)


# ---------------------------------------------------------------------------
# host-side helpers
# ---------------------------------------------------------------------------

def _f16(v):
    return np.asarray(v, np.float32).astype(np.float16)


def _host_constants(W):
    """Operand matrices from W (fp64 powers -> fp16).

    Output column index m = (C-1-j)*S + s  (reversed j)."""
    W64 = W.astype(np.float64)
    M = W64[:, :S].T            # (2, 2)
    WzT = W64[:, S:].T          # (32, 2)

    Mpow = [np.eye(S)]
    for _ in range(C + 1):
        Mpow.append(Mpow[-1] @ M)

    def col(j):
        return (C - 1 - j) * S

    Q = np.zeros((C * D, C * S), np.float64)
    R = np.zeros((S, C * S), np.float64)
    for j in range(C):
        R[:, col(j):col(j) + S] = Mpow[j]
        for tl in range(j):
            Q[tl * D:(tl + 1) * D, col(j):col(j) + S] = WzT @ Mpow[j - 1 - tl]

    MR = (M @ R) / CSC          # carry is stored scaled by CSC
    QU = WzT @ R

    Rf = np.float32(R)
    R_hi = _f16(Rf)
    R_lo = _f16(Rf - R_hi)
    MRf = np.float32(MR)
    MR_hi = _f16(MRf)
    MR_lo = _f16(MRf - MR_hi)

    # QU padded to K=128: rows 96:128 (t-local 63 of the last K-tile) = WzT R
    qu = np.zeros((128, C * S), np.float16)
    qu[96:128, :] = _f16(QU)

    qmat = np.ascontiguousarray(_f16(Q).reshape(KT, 128, C * S))

    return {
        "qmat": qmat,                                  # (16, 128, 128) fp16
        "rhi": np.ascontiguousarray(R_hi),             # (2, 128)
        "rlo": np.ascontiguousarray(R_lo),             # (2, 128)
        "mrhi": np.ascontiguousarray(MR_hi),           # (2, 128)
        "mrlo": np.ascontiguousarray(MR_lo),           # (2, 128)
        "qu": qu,                                      # (128, 128)
    }


def _host_z(z, nch):
    """Pre-transpose z to (core, chunk, td, b) fp16."""
    zc = z[:, :nch * C, :]
    zt = _f16(zc).reshape(NCORES, BL, nch, C * D)      # (core, b, chunk, td)
    zt = np.ascontiguousarray(zt.transpose(0, 2, 3, 1))  # (core, chunk, td, b)
    return zt


# ---------------------------------------------------------------------------
# workarounds for this container's walrus (max 1 sem-wait per instruction)
# ---------------------------------------------------------------------------

def _install_patches():
    import concourse.tile as tile
    import concourse.mybir as mybir
    from bass_rust import ScopedClock

    if getattr(tile.TileContext, "_ard_patched", False):
        return

    def _drain_and_barrier(self, tick_clock, wait_clock):
        nc = self.nc
        probe = nc.sync.nop(nofuse=True, hint="tail_wait_spread")
        wait_clock.add_sem_waits(
            probe.ins, ScopedClock({None: tick_clock.global_clock})
        )
        si = probe.ins.sync_info
        waits = list(si.on_wait) if si is not None else []
        updates = list(si.on_update) if si is not None else []
        if len(waits) > 1:
            probe.ins.sync_info = mybir.SyncInfo(on_wait=waits[:1], on_update=updates)
            for w in waits[1:]:
                n2 = nc.sync.nop(nofuse=True, hint="tail_wait_spread")
                n2.ins.sync_info = mybir.SyncInfo(on_wait=[w], on_update=[])
        nc.sync.drain()
        nc.all_engine_barrier()
        assert self.sems is not None
        popped = nc._tile_sem_poison_stack.pop()
        assert popped is self._sem_poison
        nc.clear_and_free_semaphores(list(self.sems.allocated().values()))
        nc.all_engine_barrier()

    tile.TileContext._drain_and_barrier = _drain_and_barrier
    tile.TileContext._ard_patched = True


def _spread_waits(nc):
    """Move excess sem-waits (>1) onto same-engine NoOps inserted just before
    the owning instruction (engines are in-order, so semantics hold)."""
    import concourse.mybir as mybir

    ctr = 0
    for f in nc.m.functions:
        for b in f.blocks:
            out = []
            changed = False
            for inst in b.instructions:
                si = inst.sync_info
                waits = list(si.on_wait) if si is not None else []
                if len(waits) > 1 and inst.engine != mybir.EngineType.Unassigned:
                    changed = True
                    for w in waits[:-1]:
                        ctr += 1
                        out.append(
                            mybir.InstNoOp(
                                name=f"waitspread-{ctr}",
                                sync_info=mybir.SyncInfo(on_wait=[w], on_update=[]),
                                bass_nofuse=True,
                                engine=inst.engine,
                            )
                        )
                    inst.sync_info = mybir.SyncInfo(
                        on_wait=waits[-1:], on_update=list(si.on_update)
                    )
                out.append(inst)
            if changed:
                b.instructions = out
    return ctr


# ---------------------------------------------------------------------------
# device program
# ---------------------------------------------------------------------------

def _build_nc(nch):
    import concourse.bass as bass
    import concourse.tile as tile
    import concourse.mybir as mybir

    _install_patches()
    f16 = mybir.dt.float16
    f32 = mybir.dt.float32
    PSUM = bass.MemorySpace.PSUM
    AluOp = mybir.AluOpType

    nc = bass.Bass(trn_type="TRN2", target_bir_lowering=False, debug=False)
    zin = nc.dram_tensor("zin", [nch, C * D, BL], f16, kind="ExternalInput")
    qmat = nc.dram_tensor("qmat", [KT, 128, C * S], f16, kind="ExternalInput")
    rhi = nc.dram_tensor("rhi", [2, C * S], f16, kind="ExternalInput")
    rlo = nc.dram_tensor("rlo", [2, C * S], f16, kind="ExternalInput")
    mrhi = nc.dram_tensor("mrhi", [2, C * S], f16, kind="ExternalInput")
    mrlo = nc.dram_tensor("mrlo", [2, C * S], f16, kind="ExternalInput")
    qu = nc.dram_tensor("qu", [128, C * S], f16, kind="ExternalInput")
    c0hi = nc.dram_tensor("c0hi", [2, BL], f16, kind="ExternalInput")
    c0lo = nc.dram_tensor("c0lo", [2, BL], f16, kind="ExternalInput")
    out = nc.dram_tensor("out", [128, nch * BL], f32, kind="ExternalOutput")

    with tile.TileContext(nc) as tc:
        with (
            tc.tile_pool(name="const", bufs=1) as const,
            tc.tile_pool(name="zbuf", bufs=8) as zbuf,
            tc.tile_pool(name="obuf", bufs=2) as obuf,
            tc.tile_pool(name="cbuf", bufs=3) as cbuf,
            tc.tile_pool(name="outps", bufs=4, space=PSUM) as outps,
        ):
            # constants go through the ACT HWDGE ring so they stream in
            # parallel with the z loads on the SP ring
            qsb = const.tile([128, KT * C * S], f16)
            nc.scalar.dma_start(
                qsb[:].rearrange("p (k m) -> p k m", k=KT),
                qmat.ap().rearrange("k p m -> p k m"),
            )
            rhisb = const.tile([2, C * S], f16)
            nc.scalar.dma_start(rhisb[:], rhi.ap())
            rlosb = const.tile([2, C * S], f16)
            nc.scalar.dma_start(rlosb[:], rlo.ap())
            mrhisb = const.tile([2, C * S], f16)
            nc.scalar.dma_start(mrhisb[:], mrhi.ap())
            mrlosb = const.tile([2, C * S], f16)
            nc.scalar.dma_start(mrlosb[:], mrlo.ap())
            qusb = const.tile([128, C * S], f16)
            nc.scalar.dma_start(qusb[:], qu.ap())
            chi0 = const.tile([2, BL], f16)
            nc.scalar.dma_start(chi0[:], c0hi.ap())
            clo0 = const.tile([2, BL], f16)
            nc.scalar.dma_start(clo0[:], c0lo.ap())

            def dma_z(k):
                zt = zbuf.tile([128, KT * BL], f16, tag="z", name="zt")
                nc.sync.dma_start(
                    zt[:].rearrange("p (kt b) -> p kt b", kt=KT),
                    zin.ap()[k].rearrange("(kt p) b -> p kt b", p=128),
                )
                return zt

            zs = {k: dma_z(k) for k in range(min(nch, 6))}

            obs = {}
            shilo = {}
            for k in range(nch):
                if k + 6 < nch:
                    zs[k + 6] = dma_z(k + 6)
                zk = zs[k]

                pout = outps.tile([128, BL], f32, tag="pout", name="pout")
                for kt in range(KT):
                    nc.tensor.matmul(
                        pout[:], qsb[:, kt * 128:(kt + 1) * 128],
                        zk[:, kt * BL:(kt + 1) * BL],
                        start=(kt == 0), stop=False,
                    )
                # carry-add (fused: previous chunk's last state + last z step)
                if k == 0:
                    nc.tensor.matmul(pout[:], rhisb[:], chi0[:], start=False, stop=False)
                    nc.tensor.matmul(pout[:], rhisb[:], clo0[:], start=False, stop=False)
                    nc.tensor.matmul(pout[:], rlosb[:], chi0[:], start=False, stop=True)
                else:
                    shi, slo = shilo[k - 1]
                    zprev = zs[k - 1]
                    nc.tensor.matmul(pout[:], mrhisb[:], shi[:], start=False, stop=False)
                    nc.tensor.matmul(pout[:], mrhisb[:], slo[:], start=False, stop=False)
                    nc.tensor.matmul(pout[:], mrlosb[:], shi[:], start=False, stop=False)
                    nc.tensor.matmul(
                        pout[:], qusb[:], zprev[:, (KT - 1) * BL:KT * BL],
                        start=False, stop=True,
                    )
                    del zs[k - 1]

                # carry state for the next chunk: scaled hi/lo split of the
                # last state (psum rows 0:2, thanks to reversed j order)
                if k < nch - 1:
                    shi = cbuf.tile([2, BL], f16, tag="shi", name="shi")
                    nc.scalar.mul(shi[:], pout[0:2, :], CSC)
                    slo = cbuf.tile([2, BL], f16, tag="slo", name="slo")
                    nc.vector.scalar_tensor_tensor(
                        slo[:], pout[0:2, :], CSC, shi[:],
                        op0=AluOp.mult, op1=AluOp.subtract,
                    )
                    shilo[k] = (shi, slo)

                # stage + write out every 2 chunks
                if k % 2 == 0:
                    obs[k // 2] = obuf.tile([128, 2 * BL], f32, tag="ob", name="ob")
                ob = obs[k // 2]
                nc.vector.tensor_copy(ob[:, (k % 2) * BL:(k % 2) * BL + BL], pout[:])
                if k % 2 == 1:
                    g = k // 2
                    nc.sync.dma_start(
                        out.ap()[:, g * 2 * BL:(g + 1) * 2 * BL], ob[:]
                    )

    _spread_waits(nc)
    return nc


_CACHE = {}


def _get_nc(nch):
    if nch not in _CACHE:
        _CACHE[nch] = _build_nc(nch)
    return _CACHE[nch]


# ---------------------------------------------------------------------------
# entry point
# ---------------------------------------------------------------------------

def _run(init_states, z, W, nch, core_ids, trace=False):
    from concourse.bass_utils import run_bass_kernel_spmd

    consts = _host_constants(W)
    zt = _host_z(np.asarray(z), nch)
    ncores = len(core_ids)
    in_maps = []
    for i in range(ncores):
        sl = slice(i * BL, (i + 1) * BL)
        init_T = np.ascontiguousarray(init_states[sl].T, np.float32)  # (2, BL)
        hi = _f16(init_T)
        lo = _f16(init_T - hi)
        in_maps.append({
            "zin": zt[i],
            "qmat": consts["qmat"],
            "rhi": consts["rhi"],
            "rlo": consts["rlo"],
            "mrhi": consts["mrhi"],
            "mrlo": consts["mrlo"],
            "qu": consts["qu"],
            "c0hi": hi,
            "c0lo": lo,
        })

    nc = _get_nc(nch)
    kwargs = {}
    if trace:
        kwargs = dict(trace=True, trace_cores=list(core_ids))
    res = run_bass_kernel_spmd(nc, in_maps, core_ids=list(core_ids), **kwargs)

    outs = []
    for i in range(ncores):
        o = res.results[i]["out"]                       # (128, nch*BL)
        o = o.reshape(C, S, nch, BL)                    # (rev_j, s, k, b)
        o = o[::-1]                                     # undo reversed j
        o = np.transpose(o, (3, 2, 0, 1)).reshape(BL, nch * C, S)
        outs.append(o)
    full = np.concatenate(outs, axis=0).astype(np.float32)
    return full, res


def kernel(init_states, z, W):
    full, _ = _run(init_states, z, W, T // C, list(range(NCORES)))
    return full


# revision 23
# speedup vs baseline: 2.2790x; 1.1494x over previous
"""Trainium2 Bass kernel for nn_AutoregressiveDecoder.

Reference computation (B=2048, T=1024, D=32, S=2):
    s_{t+1} = s_t @ Ws.T + z_t @ Wz.T        (Ws = W[:, :2], Wz = W[:, 2:])
    out[:, t] = s_t,  s_0 = init_states      -> (B, T, S) fp32

Strategy: data-parallel over 8 cores (256 batch rows each). The sequential
scan is re-expressed as 16 chunks of C=64 steps; within a chunk all 64
outputs are produced by ONE PE accumulation group against a host-precomputed
block-Toeplitz matrix Q[(tl,d),(j,s)] = (Wz^T M^{j-1-tl})[d,s] (M = Ws^T),
plus a carry-in term. The inter-chunk carry is fused into the next chunk's
accumulation:  O_{k+1} += s_last(k)·(M R) + z_last(k)·(Wz^T R),  where
R[(s'),(j,s)] = M^j, so there is no separate carry matmul group at all.
The carry state is hi/lo-split in fp16 (scaled by 1/16 against overflow) so
the 16-step chain keeps fp32-level accuracy.

z is pre-transposed AND pre-converted to fp16 on the host (m10 ~ the PE's
native f32r m11 precision, half the HBM traffic), so the kernel needs no
on-chip transposes — all PE work is plain fp16 matmuls, which also keeps
the PE HAM clock-gate warm (transpose-mode ops do not count as PE activity
and previously kept the clock throttled at 1.2 GHz).

Output rows use REVERSED j order (row = (C-1-j)*S + s) so the last state of
a chunk lands on partitions 0:2 (partition bases must be 32-aligned).
"""

import numpy as np

B, T, D, S = 2048, 1024, 32, 2
C = 64                  # time steps per chunk
NCORES = 8
BL = B // NCORES        # 256 batch rows per core
KT = C * D // 128       # 16 K-tiles of 128 per chunk
CSC = 1.0 / 16.0        # carry scale (power of two; MR is stored x16)


# ---------------------------------------------------------------------------
# host-side helpers
# ---------------------------------------------------------------------------

def _f16(v):
    return np.asarray(v, np.float32).astype(np.float16)


def _host_constants(W):
    """Operand matrices from W (fp64 powers -> fp16).

    Output column index m = (C-1-j)*S + s  (reversed j)."""
    W64 = W.astype(np.float64)
    M = W64[:, :S].T            # (2, 2)
    WzT = W64[:, S:].T          # (32, 2)

    Mpow = [np.eye(S)]
    for _ in range(C + 1):
        Mpow.append(Mpow[-1] @ M)

    def col(j):
        return (C - 1 - j) * S

    Q = np.zeros((C * D, C * S), np.float64)
    R = np.zeros((S, C * S), np.float64)
    for j in range(C):
        R[:, col(j):col(j) + S] = Mpow[j]
        for tl in range(j):
            Q[tl * D:(tl + 1) * D, col(j):col(j) + S] = WzT @ Mpow[j - 1 - tl]

    MR = (M @ R) / CSC          # carry is stored scaled by CSC
    QU = WzT @ R

    Rf = np.float32(R)
    R_hi = _f16(Rf)
    R_lo = _f16(Rf - R_hi)
    MRf = np.float32(MR)
    MR_hi = _f16(MRf)
    MR_lo = _f16(MRf - MR_hi)

    # QU padded to K=128: rows 96:128 (t-local 63 of the last K-tile) = WzT R
    qu = np.zeros((128, C * S), np.float16)
    qu[96:128, :] = _f16(QU)

    # swizzled so each SBUF partition's data is one contiguous run
    qmat = np.ascontiguousarray(
        _f16(Q).reshape(KT, 128, C * S).transpose(1, 0, 2))  # (p, kt, m)

    return {
        "qmat": qmat,                                  # (128, 16, 128) fp16
        "rhi": np.ascontiguousarray(R_hi),             # (2, 128)
        "rlo": np.ascontiguousarray(R_lo),             # (2, 128)
        "mrhi": np.ascontiguousarray(MR_hi),           # (2, 128)
        "mrlo": np.ascontiguousarray(MR_lo),           # (2, 128)
        "qu": qu,                                      # (128, 128)
    }


def _host_z(z, nch):
    """Pre-transpose z to (core, chunk, p, kt, b) fp16 — swizzled so each
    SBUF partition's chunk data is one contiguous 8KB run."""
    zc = z[:, :nch * C, :]
    zt = _f16(zc).reshape(NCORES, BL, nch, KT, 128)    # (core, b, chunk, kt, p)
    zt = np.ascontiguousarray(zt.transpose(0, 2, 4, 3, 1))  # (core, chunk, p, kt, b)
    return zt.reshape(NCORES, nch, 128, KT * BL)


# ---------------------------------------------------------------------------
# workarounds for this container's walrus (max 1 sem-wait per instruction)
# ---------------------------------------------------------------------------

def _install_patches():
    import concourse.tile as tile
    import concourse.mybir as mybir
    from bass_rust import ScopedClock

    if getattr(tile.TileContext, "_ard_patched", False):
        return

    def _drain_and_barrier(self, tick_clock, wait_clock):
        nc = self.nc
        probe = nc.sync.nop(nofuse=True, hint="tail_wait_spread")
        wait_clock.add_sem_waits(
            probe.ins, ScopedClock({None: tick_clock.global_clock})
        )
        si = probe.ins.sync_info
        waits = list(si.on_wait) if si is not None else []
        updates = list(si.on_update) if si is not None else []
        if len(waits) > 1:
            probe.ins.sync_info = mybir.SyncInfo(on_wait=waits[:1], on_update=updates)
            for w in waits[1:]:
                n2 = nc.sync.nop(nofuse=True, hint="tail_wait_spread")
                n2.ins.sync_info = mybir.SyncInfo(on_wait=[w], on_update=[])
        nc.sync.drain()
        nc.all_engine_barrier()
        assert self.sems is not None
        popped = nc._tile_sem_poison_stack.pop()
        assert popped is self._sem_poison
        nc.clear_and_free_semaphores(list(self.sems.allocated().values()))
        nc.all_engine_barrier()

    tile.TileContext._drain_and_barrier = _drain_and_barrier
    tile.TileContext._ard_patched = True


def _spread_waits(nc):
    """Move excess sem-waits (>1) onto same-engine NoOps inserted just before
    the owning instruction (engines are in-order, so semantics hold)."""
    import concourse.mybir as mybir

    ctr = 0
    for f in nc.m.functions:
        for b in f.blocks:
            out = []
            changed = False
            for inst in b.instructions:
                si = inst.sync_info
                waits = list(si.on_wait) if si is not None else []
                if len(waits) > 1 and inst.engine != mybir.EngineType.Unassigned:
                    changed = True
                    for w in waits[:-1]:
                        ctr += 1
                        out.append(
                            mybir.InstNoOp(
                                name=f"waitspread-{ctr}",
                                sync_info=mybir.SyncInfo(on_wait=[w], on_update=[]),
                                bass_nofuse=True,
                                engine=inst.engine,
                            )
                        )
                    inst.sync_info = mybir.SyncInfo(
                        on_wait=waits[-1:], on_update=list(si.on_update)
                    )
                out.append(inst)
            if changed:
                b.instructions = out
    return ctr


# ---------------------------------------------------------------------------
# device program
# ---------------------------------------------------------------------------

def _build_nc(nch):
    import concourse.bass as bass
    import concourse.tile as tile
    import concourse.mybir as mybir

    _install_patches()
    f16 = mybir.dt.float16
    f32 = mybir.dt.float32
    PSUM = bass.MemorySpace.PSUM
    AluOp = mybir.AluOpType

    nc = bass.Bass(trn_type="TRN2", target_bir_lowering=False, debug=False)
    zin = nc.dram_tensor("zin", [nch, 128, KT * BL], f16, kind="ExternalInput")
    qmat = nc.dram_tensor("qmat", [128, KT * C * S], f16, kind="ExternalInput")
    rhi = nc.dram_tensor("rhi", [2, C * S], f16, kind="ExternalInput")
    rlo = nc.dram_tensor("rlo", [2, C * S], f16, kind="ExternalInput")
    mrhi = nc.dram_tensor("mrhi", [2, C * S], f16, kind="ExternalInput")
    mrlo = nc.dram_tensor("mrlo", [2, C * S], f16, kind="ExternalInput")
    qu = nc.dram_tensor("qu", [128, C * S], f16, kind="ExternalInput")
    c0hi = nc.dram_tensor("c0hi", [2, BL], f16, kind="ExternalInput")
    c0lo = nc.dram_tensor("c0lo", [2, BL], f16, kind="ExternalInput")
    out = nc.dram_tensor("out", [128, nch * BL], f32, kind="ExternalOutput")

    with tile.TileContext(nc) as tc:
        with (
            tc.tile_pool(name="const", bufs=1) as const,
            tc.tile_pool(name="zbuf", bufs=8) as zbuf,
            tc.tile_pool(name="obuf", bufs=2) as obuf,
            tc.tile_pool(name="cbuf", bufs=3) as cbuf,
            tc.tile_pool(name="outps", bufs=4, space=PSUM) as outps,
        ):
            # constants go through the ACT HWDGE ring so they stream in
            # parallel with the z loads on the SP ring
            qsb = const.tile([128, KT * C * S], f16)
            nc.scalar.dma_start(qsb[:], qmat.ap())
            rhisb = const.tile([2, C * S], f16)
            nc.scalar.dma_start(rhisb[:], rhi.ap())
            rlosb = const.tile([2, C * S], f16)
            nc.scalar.dma_start(rlosb[:], rlo.ap())
            mrhisb = const.tile([2, C * S], f16)
            nc.scalar.dma_start(mrhisb[:], mrhi.ap())
            mrlosb = const.tile([2, C * S], f16)
            nc.scalar.dma_start(mrlosb[:], mrlo.ap())
            qusb = const.tile([128, C * S], f16)
            nc.scalar.dma_start(qusb[:], qu.ap())
            chi0 = const.tile([2, BL], f16)
            nc.scalar.dma_start(chi0[:], c0hi.ap())
            clo0 = const.tile([2, BL], f16)
            nc.scalar.dma_start(clo0[:], c0lo.ap())

            def dma_z(k):
                zt = zbuf.tile([128, KT * BL], f16, tag="z", name="zt")
                nc.sync.dma_start(zt[:], zin.ap()[k])
                return zt

            zs = {k: dma_z(k) for k in range(min(nch, 6))}

            obs = {}
            shilo = {}
            for k in range(nch):
                if k + 6 < nch:
                    zs[k + 6] = dma_z(k + 6)
                zk = zs[k]

                pout = outps.tile([128, BL], f32, tag="pout", name="pout")
                for kt in range(KT):
                    nc.tensor.matmul(
                        pout[:], qsb[:, kt * 128:(kt + 1) * 128],
                        zk[:, kt * BL:(kt + 1) * BL],
                        start=(kt == 0), stop=False,
                    )
                # carry-add (fused: previous chunk's last state + last z step)
                if k == 0:
                    nc.tensor.matmul(pout[:], rhisb[:], chi0[:], start=False, stop=False)
                    nc.tensor.matmul(pout[:], rhisb[:], clo0[:], start=False, stop=False)
                    nc.tensor.matmul(pout[:], rlosb[:], chi0[:], start=False, stop=True)
                else:
                    shi, slo = shilo[k - 1]
                    zprev = zs[k - 1]
                    nc.tensor.matmul(pout[:], mrhisb[:], shi[:], start=False, stop=False)
                    nc.tensor.matmul(pout[:], mrhisb[:], slo[:], start=False, stop=False)
                    nc.tensor.matmul(pout[:], mrlosb[:], shi[:], start=False, stop=False)
                    nc.tensor.matmul(
                        pout[:], qusb[:], zprev[:, (KT - 1) * BL:KT * BL],
                        start=False, stop=True,
                    )
                    del zs[k - 1]

                # carry state for the next chunk: scaled hi/lo split of the
                # last state (psum rows 0:2, thanks to reversed j order)
                if k < nch - 1:
                    shi = cbuf.tile([2, BL], f16, tag="shi", name="shi")
                    nc.scalar.mul(shi[:], pout[0:2, :], CSC)
                    slo = cbuf.tile([2, BL], f16, tag="slo", name="slo")
                    nc.vector.scalar_tensor_tensor(
                        slo[:], pout[0:2, :], CSC, shi[:],
                        op0=AluOp.mult, op1=AluOp.subtract,
                    )
                    shilo[k] = (shi, slo)

                # stage + write out every 2 chunks
                if k % 2 == 0:
                    obs[k // 2] = obuf.tile([128, 2 * BL], f32, tag="ob", name="ob")
                ob = obs[k // 2]
                nc.vector.tensor_copy(ob[:, (k % 2) * BL:(k % 2) * BL + BL], pout[:])
                if k % 2 == 1:
                    g = k // 2
                    nc.sync.dma_start(
                        out.ap()[:, g * 2 * BL:(g + 1) * 2 * BL], ob[:]
                    )

    _spread_waits(nc)
    return nc


_CACHE = {}


def _get_nc(nch):
    if nch not in _CACHE:
        _CACHE[nch] = _build_nc(nch)
    return _CACHE[nch]


# ---------------------------------------------------------------------------
# entry point
# ---------------------------------------------------------------------------

def _run(init_states, z, W, nch, core_ids, trace=False):
    from concourse.bass_utils import run_bass_kernel_spmd

    consts = _host_constants(W)
    zt = _host_z(np.asarray(z), nch)
    ncores = len(core_ids)
    in_maps = []
    for i in range(ncores):
        sl = slice(i * BL, (i + 1) * BL)
        init_T = np.ascontiguousarray(init_states[sl].T, np.float32)  # (2, BL)
        hi = _f16(init_T)
        lo = _f16(init_T - hi)
        in_maps.append({
            "zin": zt[i],
            "qmat": consts["qmat"],
            "rhi": consts["rhi"],
            "rlo": consts["rlo"],
            "mrhi": consts["mrhi"],
            "mrlo": consts["mrlo"],
            "qu": consts["qu"],
            "c0hi": hi,
            "c0lo": lo,
        })

    nc = _get_nc(nch)
    kwargs = {}
    if trace:
        kwargs = dict(trace=True, trace_cores=list(core_ids))
    res = run_bass_kernel_spmd(nc, in_maps, core_ids=list(core_ids), **kwargs)

    outs = []
    for i in range(ncores):
        o = res.results[i]["out"]                       # (128, nch*BL)
        o = o.reshape(C, S, nch, BL)                    # (rev_j, s, k, b)
        o = o[::-1]                                     # undo reversed j
        o = np.transpose(o, (3, 2, 0, 1)).reshape(BL, nch * C, S)
        outs.append(o)
    full = np.concatenate(outs, axis=0).astype(np.float32)
    return full, res


def kernel(init_states, z, W):
    full, _ = _run(init_states, z, W, T // C, list(range(NCORES)))
    return full


# revision 27
# speedup vs baseline: 2.2837x; 1.0021x over previous
"""Trainium2 Bass kernel for nn_AutoregressiveDecoder.

Reference computation (B=2048, T=1024, D=32, S=2):
    s_{t+1} = s_t @ Ws.T + z_t @ Wz.T        (Ws = W[:, :2], Wz = W[:, 2:])
    out[:, t] = s_t,  s_0 = init_states      -> (B, T, S) fp32

Strategy: data-parallel over 8 cores (256 batch rows each). The sequential
scan is re-expressed as 16 chunks of C=64 steps; within a chunk all 64
outputs are produced by ONE PE accumulation group against a host-precomputed
block-Toeplitz matrix Q[(tl,d),(j,s)] = (Wz^T M^{j-1-tl})[d,s] (M = Ws^T),
plus a carry-in term. The inter-chunk carry is fused into the next chunk's
accumulation:  O_{k+1} += s_last(k)·(M R) + z_last(k)·(Wz^T R),  where
R[(s'),(j,s)] = M^j, so there is no separate carry matmul group at all.
The carry state is hi/lo-split in fp16 (scaled by 1/16 against overflow) so
the 16-step chain keeps fp32-level accuracy.

z is pre-transposed AND pre-converted to fp16 on the host (m10 ~ the PE's
native f32r m11 precision, half the HBM traffic), so the kernel needs no
on-chip transposes — all PE work is plain fp16 matmuls, which also keeps
the PE HAM clock-gate warm (transpose-mode ops do not count as PE activity
and previously kept the clock throttled at 1.2 GHz).

Output rows use REVERSED j order (row = (C-1-j)*S + s) so the last state of
a chunk lands on partitions 0:2 (partition bases must be 32-aligned).
"""

import numpy as np

B, T, D, S = 2048, 1024, 32, 2
C = 64                  # time steps per chunk
NCORES = 8
BL = B // NCORES        # 256 batch rows per core
KT = C * D // 128       # 16 K-tiles of 128 per chunk
CSC = 1.0 / 16.0        # carry scale (power of two; MR is stored x16)


# ---------------------------------------------------------------------------
# host-side helpers
# ---------------------------------------------------------------------------

def _f16(v):
    return np.asarray(v, np.float32).astype(np.float16)


def _host_constants(W):
    """Operand matrices from W (fp64 powers -> fp16).

    Output column index m = (C-1-j)*S + s  (reversed j)."""
    W64 = W.astype(np.float64)
    M = W64[:, :S].T            # (2, 2)
    WzT = W64[:, S:].T          # (32, 2)

    Mpow = [np.eye(S)]
    for _ in range(C + 1):
        Mpow.append(Mpow[-1] @ M)

    def col(j):
        return (C - 1 - j) * S

    Q = np.zeros((C * D, C * S), np.float64)
    R = np.zeros((S, C * S), np.float64)
    for j in range(C):
        R[:, col(j):col(j) + S] = Mpow[j]
        for tl in range(j):
            Q[tl * D:(tl + 1) * D, col(j):col(j) + S] = WzT @ Mpow[j - 1 - tl]

    MR = (M @ R) / CSC          # carry is stored scaled by CSC
    QU = WzT @ R

    Rf = np.float32(R)
    R_hi = _f16(Rf)
    R_lo = _f16(Rf - R_hi)
    MRf = np.float32(MR)
    MR_hi = _f16(MRf)
    MR_lo = _f16(MRf - MR_hi)

    # QU padded to K=128: rows 96:128 (t-local 63 of the last K-tile) = WzT R
    qu = np.zeros((128, C * S), np.float16)
    qu[96:128, :] = _f16(QU)

    # swizzled so each SBUF partition's data is one contiguous run
    qmat = np.ascontiguousarray(
        _f16(Q).reshape(KT, 128, C * S).transpose(1, 0, 2))  # (p, kt, m)

    return {
        "qmat": qmat,                                  # (128, 16, 128) fp16
        "rhi": np.ascontiguousarray(R_hi),             # (2, 128)
        "rlo": np.ascontiguousarray(R_lo),             # (2, 128)
        "mrhi": np.ascontiguousarray(MR_hi),           # (2, 128)
        "mrlo": np.ascontiguousarray(MR_lo),           # (2, 128)
        "qu": qu,                                      # (128, 128)
    }


def _host_z(z, nch):
    """Pre-transpose z to (core, chunk, p, kt, b) fp16 — swizzled so each
    SBUF partition's chunk data is one contiguous 8KB run."""
    zc = z[:, :nch * C, :]
    zt = _f16(zc).reshape(NCORES, BL, nch, KT, 128)    # (core, b, chunk, kt, p)
    zt = np.ascontiguousarray(zt.transpose(0, 2, 4, 3, 1))  # (core, chunk, p, kt, b)
    return zt.reshape(NCORES, nch, 128, KT * BL)


# ---------------------------------------------------------------------------
# workarounds for this container's walrus (max 1 sem-wait per instruction)
# ---------------------------------------------------------------------------

def _install_patches():
    import concourse.tile as tile
    import concourse.mybir as mybir
    from bass_rust import ScopedClock

    if getattr(tile.TileContext, "_ard_patched", False):
        return

    def _drain_and_barrier(self, tick_clock, wait_clock):
        nc = self.nc
        probe = nc.sync.nop(nofuse=True, hint="tail_wait_spread")
        wait_clock.add_sem_waits(
            probe.ins, ScopedClock({None: tick_clock.global_clock})
        )
        si = probe.ins.sync_info
        waits = list(si.on_wait) if si is not None else []
        updates = list(si.on_update) if si is not None else []
        if len(waits) > 1:
            probe.ins.sync_info = mybir.SyncInfo(on_wait=waits[:1], on_update=updates)
            for w in waits[1:]:
                n2 = nc.sync.nop(nofuse=True, hint="tail_wait_spread")
                n2.ins.sync_info = mybir.SyncInfo(on_wait=[w], on_update=[])
        nc.sync.drain()
        nc.all_engine_barrier()
        assert self.sems is not None
        popped = nc._tile_sem_poison_stack.pop()
        assert popped is self._sem_poison
        nc.clear_and_free_semaphores(list(self.sems.allocated().values()))
        nc.all_engine_barrier()

    tile.TileContext._drain_and_barrier = _drain_and_barrier
    tile.TileContext._ard_patched = True


def _spread_waits(nc):
    """Move excess sem-waits (>1) onto same-engine NoOps inserted just before
    the owning instruction (engines are in-order, so semantics hold)."""
    import concourse.mybir as mybir

    ctr = 0
    for f in nc.m.functions:
        for b in f.blocks:
            out = []
            changed = False
            for inst in b.instructions:
                si = inst.sync_info
                waits = list(si.on_wait) if si is not None else []
                if len(waits) > 1 and inst.engine != mybir.EngineType.Unassigned:
                    changed = True
                    for w in waits[:-1]:
                        ctr += 1
                        out.append(
                            mybir.InstNoOp(
                                name=f"waitspread-{ctr}",
                                sync_info=mybir.SyncInfo(on_wait=[w], on_update=[]),
                                bass_nofuse=True,
                                engine=inst.engine,
                            )
                        )
                    inst.sync_info = mybir.SyncInfo(
                        on_wait=waits[-1:], on_update=list(si.on_update)
                    )
                out.append(inst)
            if changed:
                b.instructions = out
    return ctr


# ---------------------------------------------------------------------------
# device program
# ---------------------------------------------------------------------------

def _build_nc(nch):
    import concourse.bass as bass
    import concourse.tile as tile
    import concourse.mybir as mybir

    _install_patches()
    f16 = mybir.dt.float16
    f32 = mybir.dt.float32
    PSUM = bass.MemorySpace.PSUM
    AluOp = mybir.AluOpType

    nc = bass.Bass(trn_type="TRN2", target_bir_lowering=False, debug=False)
    zin = nc.dram_tensor("zin", [nch, 128, KT * BL], f16, kind="ExternalInput")
    qmat = nc.dram_tensor("qmat", [128, KT * C * S], f16, kind="ExternalInput")
    rhi = nc.dram_tensor("rhi", [2, C * S], f16, kind="ExternalInput")
    rlo = nc.dram_tensor("rlo", [2, C * S], f16, kind="ExternalInput")
    mrhi = nc.dram_tensor("mrhi", [2, C * S], f16, kind="ExternalInput")
    mrlo = nc.dram_tensor("mrlo", [2, C * S], f16, kind="ExternalInput")
    qu = nc.dram_tensor("qu", [128, C * S], f16, kind="ExternalInput")
    c0hi = nc.dram_tensor("c0hi", [2, BL], f16, kind="ExternalInput")
    c0lo = nc.dram_tensor("c0lo", [2, BL], f16, kind="ExternalInput")
    out = nc.dram_tensor("out", [128, nch * BL], f32, kind="ExternalOutput")

    with tile.TileContext(nc) as tc:
        with (
            tc.tile_pool(name="const", bufs=1) as const,
            tc.tile_pool(name="zbuf", bufs=9) as zbuf,
            tc.tile_pool(name="obuf", bufs=2) as obuf,
            tc.tile_pool(name="cbuf", bufs=3) as cbuf,
            tc.tile_pool(name="outps", bufs=4, space=PSUM) as outps,
        ):
            # constants go through the ACT HWDGE ring so they stream in
            # parallel with the z loads on the SP ring
            qsb = const.tile([128, KT * C * S], f16)
            for q4 in range(4):
                w = KT * C * S // 4
                nc.scalar.dma_start(
                    qsb[:, q4 * w:(q4 + 1) * w], qmat.ap()[:, q4 * w:(q4 + 1) * w]
                )
            rhisb = const.tile([2, C * S], f16)
            nc.scalar.dma_start(rhisb[:], rhi.ap())
            rlosb = const.tile([2, C * S], f16)
            nc.scalar.dma_start(rlosb[:], rlo.ap())
            mrhisb = const.tile([2, C * S], f16)
            nc.scalar.dma_start(mrhisb[:], mrhi.ap())
            mrlosb = const.tile([2, C * S], f16)
            nc.scalar.dma_start(mrlosb[:], mrlo.ap())
            qusb = const.tile([128, C * S], f16)
            nc.scalar.dma_start(qusb[:], qu.ap())
            chi0 = const.tile([2, BL], f16)
            nc.scalar.dma_start(chi0[:], c0hi.ap())
            clo0 = const.tile([2, BL], f16)
            nc.scalar.dma_start(clo0[:], c0lo.ap())

            def dma_z(k, nsplit=1):
                zt = zbuf.tile([128, KT * BL], f16, tag="z", name="zt")
                w = KT * BL // nsplit
                for h in range(nsplit):
                    nc.sync.dma_start(
                        zt[:, h * w:(h + 1) * w], zin.ap()[k][:, h * w:(h + 1) * w]
                    )
                return zt

            zs = {k: dma_z(k, nsplit=(4 if k < 2 else 1))
                  for k in range(min(nch, 7))}

            obs = {}
            shilo = {}
            for k in range(nch):
                if k + 7 < nch:
                    zs[k + 7] = dma_z(k + 7)
                zk = zs[k]

                pout = outps.tile([128, BL], f32, tag="pout", name="pout")
                for kt in range(KT):
                    nc.tensor.matmul(
                        pout[:], qsb[:, kt * 128:(kt + 1) * 128],
                        zk[:, kt * BL:(kt + 1) * BL],
                        start=(kt == 0), stop=False,
                    )
                # carry-add (fused: previous chunk's last state + last z step)
                if k == 0:
                    nc.tensor.matmul(pout[:], rhisb[:], chi0[:], start=False, stop=False)
                    nc.tensor.matmul(pout[:], rhisb[:], clo0[:], start=False, stop=False)
                    nc.tensor.matmul(pout[:], rlosb[:], chi0[:], start=False, stop=True)
                else:
                    shi, slo = shilo[k - 1]
                    zprev = zs[k - 1]
                    nc.tensor.matmul(pout[:], mrhisb[:], shi[:], start=False, stop=False)
                    nc.tensor.matmul(pout[:], mrhisb[:], slo[:], start=False, stop=False)
                    nc.tensor.matmul(pout[:], mrlosb[:], shi[:], start=False, stop=False)
                    nc.tensor.matmul(
                        pout[:], qusb[:], zprev[:, (KT - 1) * BL:KT * BL],
                        start=False, stop=True,
                    )
                    del zs[k - 1]

                # carry state for the next chunk: scaled hi/lo split of the
                # last state (psum rows 0:2, thanks to reversed j order)
                if k < nch - 1:
                    shi = cbuf.tile([2, BL], f16, tag="shi", name="shi")
                    nc.scalar.mul(shi[:], pout[0:2, :], CSC)
                    slo = cbuf.tile([2, BL], f16, tag="slo", name="slo")
                    nc.vector.scalar_tensor_tensor(
                        slo[:], pout[0:2, :], CSC, shi[:],
                        op0=AluOp.mult, op1=AluOp.subtract,
                    )
                    shilo[k] = (shi, slo)

                # stage + write out every 2 chunks
                if k % 2 == 0:
                    obs[k // 2] = obuf.tile([128, 2 * BL], f32, tag="ob", name="ob")
                ob = obs[k // 2]
                nc.vector.tensor_copy(ob[:, (k % 2) * BL:(k % 2) * BL + BL], pout[:])
                if k % 2 == 1:
                    g = k // 2
                    nc.sync.dma_start(
                        out.ap()[:, g * 2 * BL:(g + 1) * 2 * BL], ob[:]
                    )

    _spread_waits(nc)
    return nc


_CACHE = {}


def _get_nc(nch):
    if nch not in _CACHE:
        _CACHE[nch] = _build_nc(nch)
    return _CACHE[nch]


# ---------------------------------------------------------------------------
# entry point
# ---------------------------------------------------------------------------

def _run(init_states, z, W, nch, core_ids, trace=False):
    from concourse.bass_utils import run_bass_kernel_spmd

    consts = _host_constants(W)
    zt = _host_z(np.asarray(z), nch)
    ncores = len(core_ids)
    in_maps = []
    for i in range(ncores):
        sl = slice(i * BL, (i + 1) * BL)
        init_T = np.ascontiguousarray(init_states[sl].T, np.float32)  # (2, BL)
        hi = _f16(init_T)
        lo = _f16(init_T - hi)
        in_maps.append({
            "zin": zt[i],
            "qmat": consts["qmat"],
            "rhi": consts["rhi"],
            "rlo": consts["rlo"],
            "mrhi": consts["mrhi"],
            "mrlo": consts["mrlo"],
            "qu": consts["qu"],
            "c0hi": hi,
            "c0lo": lo,
        })

    nc = _get_nc(nch)
    kwargs = {}
    if trace:
        kwargs = dict(trace=True, trace_cores=list(core_ids))
    res = run_bass_kernel_spmd(nc, in_maps, core_ids=list(core_ids), **kwargs)

    outs = []
    for i in range(ncores):
        o = res.results[i]["out"]                       # (128, nch*BL)
        o = o.reshape(C, S, nch, BL)                    # (rev_j, s, k, b)
        o = o[::-1]                                     # undo reversed j
        o = np.transpose(o, (3, 2, 0, 1)).reshape(BL, nch * C, S)
        outs.append(o)
    full = np.concatenate(outs, axis=0).astype(np.float32)
    return full, res


def kernel(init_states, z, W):
    full, _ = _run(init_states, z, W, T // C, list(range(NCORES)))
    return full
